# revision 12
# baseline (speedup 1.0000x reference)
"""Causal self-attention Bass/Tile kernel for Trainium2 (8 NeuronCores).

Problem: y = CausalSelfAttention(x) with
  B=8, T=1024, C=1024, H=16 heads, hs=64.
  qkv = x @ W_qkv + b_qkv;  per-head causal softmax(q k^T / sqrt(hs)) @ v;
  y = out @ W_proj + b_proj.

Sharding: pure data parallel — core i computes batch element i end-to-end.
No collectives.

Per-core plan (all matmuls fp32r except P·V which is bf16):
  1. Load x[b] [T,C] natural, PE-transpose 128x128 blocks -> xT [C,T].
  2. qkT [2C,T] = (W_qk)^T x^T via matmuls (lhsT = W chunk, rhs = xT), with
     the 1/sqrt(hs) score scale pre-folded into W_q/b_q on the host.
  3. v [T,C] natural via matmuls (lhsT = xT chunk, rhs = W_v), stored bf16
     into v_pad [T, kb, h, 65] whose 65th column is ones (fused row-sum).
  4. Scores TRANSPOSED: S^T[k,q] tile = matmul(lhsT=kT chunk, rhs=qT), two
     heads packed onto PE row-groups (K=64 each) via tile_position.
     exp on ACT straight out of PSUM (no max subtraction needed; scores are
     O(1) by construction), bf16 out. Causal mask = multiplicative
     upper-triangular mask on the diagonal 128x128 block only.
  5. PV: outT[h] [65, q] += matmul(lhsT=v_pad[:,kb,h,:], rhs=P^T tiles).
     Row 64 = softmax denominator. reciprocal (DVE) -> partition_broadcast
     (GPSIMD) -> normalize during the PSUM->SBUF copy (DVE).
  6. proj: y [T,C] = matmul(lhsT=outT chunk, rhs=W_proj) + b_proj.
"""

import os
from contextlib import ExitStack

import numpy as np
import ml_dtypes

import concourse.bass as bass
import concourse.bacc as bacc
import concourse.mybir as mybir
import concourse.tile as tile
from concourse.bass_utils import run_bass_kernel_spmd

F32 = mybir.dt.float32
F32R = mybir.dt.float32r
BF16 = mybir.dt.bfloat16

P = 128
B = 8
T = 1024
C = 1024
H = 16
HS = 64
TO = T // P   # 8 t-blocks
CO = C // P   # 8 c-chunks
NPAIR = H // 2  # 8 head pairs

# module-level knobs for test.py
TRACE = bool(int(os.environ.get("KERNEL_TRACE", "0")))
LAST_RESULTS = None  # BassKernelResults of last run


def _q_chunks(kb):
    """512-aligned psum chunks of the causal q-range [kb*128, T) for key row-block kb."""
    out = []
    q0 = kb * P
    for qc in range(T // 512):
        lo = max(q0, qc * 512)
        hi = (qc + 1) * 512
        if lo < hi:
            out.append((lo, hi - lo))
    return out


def build_nc():
    nc = bacc.Bacc("TRN2", target_bir_lowering=False, debug=False)

    x_d = nc.dram_tensor("x", [T, C], F32, kind="ExternalInput").ap()
    wqk_d = nc.dram_tensor("wqk", [C, 2 * C], F32R, kind="ExternalInput").ap()
    wv_d = nc.dram_tensor("wv", [C, C], F32R, kind="ExternalInput").ap()
    wproj_d = nc.dram_tensor("wproj", [C, C], F32R, kind="ExternalInput").ap()
    bqk_d = nc.dram_tensor("bqk", [2 * C], F32, kind="ExternalInput").ap()
    bv_d = nc.dram_tensor("bv", [C], F32R, kind="ExternalInput").ap()
    bproj_d = nc.dram_tensor("bproj", [C], F32R, kind="ExternalInput").ap()
    ident_d = nc.dram_tensor("ident", [P, P], F32, kind="ExternalInput").ap()
    ones_d = nc.dram_tensor("ones", [1, P], F32R, kind="ExternalInput").ap()
    mask_d = nc.dram_tensor("mask", [P, P], BF16, kind="ExternalInput").ap()
    y_d = nc.dram_tensor("y", [T, C], F32, kind="ExternalOutput").ap()

    with tile.TileContext(nc) as tc:
        _attn_body(tc, x_d, wqk_d, wv_d, wproj_d, bqk_d, bv_d, bproj_d,
                   ident_d, ones_d, mask_d, y_d)
    nc.compile()
    return nc


def _attn_body(tc, x_d, wqk_d, wv_d, wproj_d, bqk_d, bv_d, bproj_d,
               ident_d, ones_d, mask_d, y_d):
    nc = tc.nc
    with ExitStack() as ctx:
        # ---- pools that live the whole kernel ----
        consts = ctx.enter_context(tc.tile_pool(name="consts", bufs=1))
        big = ctx.enter_context(tc.tile_pool(name="big", bufs=1))
        ps_a = ctx.enter_context(tc.tile_pool(name="ps_a", bufs=4, space="PSUM"))
        ps_mm = ctx.enter_context(tc.tile_pool(name="ps_mm", bufs=2, space="PSUM"))
        ps_pv = ctx.enter_context(tc.tile_pool(name="ps_pv", bufs=2, space="PSUM"))

        # ---- constants ----
        ident_sb = consts.tile([P, P], F32, name="ident_sb")
        nc.sync.dma_start(ident_sb, ident_d)
        mask_sb = consts.tile([P, P], BF16, name="mask_sb")
        nc.sync.dma_start(mask_sb, mask_d)
        bqk_sb = consts.tile([P, 2 * C // P], F32, name="bqk_sb")
        nc.sync.dma_start(bqk_sb, bqk_d.rearrange("(m p) -> p m", p=P))
        ones_sb = consts.tile([1, P], F32R, name="ones_sb")
        nc.sync.dma_start(ones_sb, ones_d)
        bv_row = consts.tile([1, C], F32R, name="bv_row")
        nc.sync.dma_start(bv_row, bv_d[None, :])
        bproj_row = consts.tile([1, C], F32R, name="bproj_row")
        nc.sync.dma_start(bproj_row, bproj_d[None, :])
        bv_bc = consts.tile([P, C], F32, name="bv_bc")
        bproj_bc = consts.tile([P, C], F32, name="bproj_bc")
        for n2 in range(C // 512):
            for row, dst in ((bv_row, bv_bc), (bproj_row, bproj_bc)):
                ps_b = ps_a.tile([P, 512], F32, name=f"ps_b{n2}", tag="a")
                nc.tensor.matmul(ps_b, ones_sb, row[:, n2 * 512:(n2 + 1) * 512],
                                 start=True, stop=True)
                nc.vector.tensor_copy(dst[:, n2 * 512:(n2 + 1) * 512], ps_b)

        # ---- resident activations ----
        qkT = big.tile([P, 2 * C // P, T], F32R, name="qkT")  # 64KB/part
        v_pad = big.tile([P, TO, H, HS + 1], BF16, name="v_pad")  # 16.6KB/part
        outT = big.tile([P, CO, T], F32R, name="outT")      # 32KB/part

        x_r = x_d.rearrange("(to p) c -> p to c", p=P)
        y_r = y_d.rearrange("(tb p) c -> p tb c", p=P)
        wqk_r = wqk_d.rearrange("(co p) r -> p co r", p=P)
        wv_r = wv_d.rearrange("(co p) n -> p co n", p=P)
        wproj_r = wproj_d.rearrange("(co p) n -> p co n", p=P)

        # ================= Phase T: load x, transpose to xT =================
        xT_pool = tc.alloc_tile_pool(name="xT_pool", bufs=1)
        xT = xT_pool.tile([P, CO, T], F32R, name="xT")      # 32KB/part
        with tc.tile_pool(name="xload", bufs=2) as xload:
            for q in range(4):
                x_q = xload.tile([P, 2, C], F32, name=f"x_q{q}", tag="xq")
                nc.sync.dma_start(x_q, x_r[:, 2 * q:2 * q + 2, :])
                for toq in range(2):
                    to = 2 * q + toq
                    for co in range(CO):
                        pst = ps_a.tile([P, 512], F32, name=f"pst{to}_{co}",
                                        tag="a")
                        nc.tensor.transpose(
                            pst[:, :P], x_q[:, toq, co * P:(co + 1) * P],
                            ident_sb)
                        nc.vector.tensor_copy(
                            xT[:, co, to * P:(to + 1) * P], pst[:, :P])

        # ================= Phase QKV =================
        # qkT[m] for m in 0..15: rows m*128..m*128+127 of (q|k) transposed.
        with tc.tile_pool(name="wqk_pool", bufs=3) as wqkp:
            for m in range(2 * C // P):
                w_m = wqkp.tile([P, CO, P], F32R, name=f"w_m{m}", tag="wqk")
                nc.sync.dma_start(w_m, wqk_r[:, :, m * P:(m + 1) * P])
                for n2 in range(T // 512):
                    ps = ps_mm.tile([P, 512], F32, name=f"qk_ps{m}_{n2}",
                                    tag="mm")
                    for co in range(CO):
                        nc.tensor.matmul(
                            ps,
                            w_m[:, co, :],
                            xT[:, co, n2 * 512:(n2 + 1) * 512],
                            start=(co == 0), stop=(co == CO - 1))
                    nc.vector.tensor_scalar_add(
                        qkT[:, m, n2 * 512:(n2 + 1) * 512], ps,
                        bqk_sb[:, m:m + 1])

        # v natural: v[t, c] += xT chunks (lhsT) @ W_v, bf16 out + ones col.
        nc.vector.memset(v_pad[:, :, :, HS:HS + 1], 1.0)
        with tc.tile_pool(name="wv_pool", bufs=1) as wvp:
            wv_sb = wvp.tile([P, CO, C], F32R, name="wv_sb")
            nc.sync.dma_start(wv_sb, wv_r)
            for tb in range(TO):
                for n2 in range(C // 512):
                    ps = ps_mm.tile([P, 512], F32, name=f"v_ps{tb}_{n2}",
                                    tag="mm")
                    for co in range(CO):
                        nc.tensor.matmul(
                            ps,
                            xT[:, co, tb * P:(tb + 1) * P],
                            wv_sb[:, co, n2 * 512:(n2 + 1) * 512],
                            start=(co == 0), stop=(co == CO - 1))
                    nc.vector.tensor_tensor(
                        out=v_pad[:, tb, n2 * 8:(n2 + 1) * 8, 0:HS],
                        in0=ps.rearrange("p (h d) -> p h d", d=HS),
                        in1=bv_bc[:, n2 * 512:(n2 + 1) * 512].rearrange(
                            "p (h d) -> p h d", d=HS),
                        op=mybir.AluOpType.add)

        xT_pool.release()  # x^T dead once qkv computed

        # load W_proj early so its DMA overlaps attention
        wproj_pool = ctx.enter_context(tc.tile_pool(name="wproj_pool", bufs=1))
        wproj_sb = wproj_pool.tile([P, CO, C], F32R, name="wproj_sb")
        nc.sync.dma_start(wproj_sb, wproj_r)

        # ================= Phase A: attention per head-pair =================
        # pt pool: per-kb-width transposed prob tiles, bf16
        with tc.tile_pool(name="pt_pool", bufs=2) as ptp, \
             tc.tile_pool(name="nrm", bufs=3) as nrm:
            for j in range(NPAIR):
                for hh in range(2):
                    h = 2 * j + hh
                    pb = hh * HS  # partition base of this head's q/k rows
                    m_q, m_k = j, NPAIR + j
                    # ---- scores S^T + exp + mask ----
                    pt = []
                    for kb in range(TO):
                        w = T - kb * P
                        pt_kb = ptp.tile([P, w], BF16, name=f"pt{h}_{kb}",
                                         tag=f"pt{kb}")
                        pt.append(pt_kb)
                        kT = qkT[pb:pb + HS, m_k, kb * P:(kb + 1) * P]
                        for (qs, qw) in _q_chunks(kb):
                            ps = ps_a.tile([P, 512], F32,
                                           name=f"s{h}_{kb}_{qs}", tag="a")
                            nc.tensor.matmul(
                                ps[:, :qw],
                                kT,
                                qkT[pb:pb + HS, m_q, qs:qs + qw],
                                start=True, stop=True,
                                tile_position=(pb, 0))
                            nc.scalar.activation(
                                out=pt_kb[:, qs - kb * P:qs - kb * P + qw],
                                in_=ps[:, :qw],
                                func=mybir.ActivationFunctionType.Exp)
                        # causal mask on the diagonal 128x128 block
                        nc.vector.tensor_mul(
                            pt_kb[:, 0:P], pt_kb[:, 0:P], mask_sb)
                    # ---- PV + row sums + normalize ----
                    for qc in range(T // 512):
                        ps_o = ps_pv.tile([HS + 1, 512], F32,
                                          name=f"o{h}_{qc}", tag="pv")
                        kbs = [kb for kb in range(TO) if kb * P < (qc + 1) * 512]
                        for i, kb in enumerate(kbs):
                            qlo = max(qc * 512, kb * P)
                            qhi = (qc + 1) * 512
                            nc.tensor.matmul(
                                ps_o[:, qlo - qc * 512:512],
                                v_pad[:, kb, h, :],
                                pt[kb][:, qlo - kb * P:qhi - kb * P],
                                start=(i == 0), stop=(i == len(kbs) - 1))
                        rec = nrm.tile([1, 512], F32R, name=f"rec{h}_{qc}",
                                       tag="rec")
                        with nc.allow_low_precision(
                                reason="f32r reciprocal is fp32-width"):
                            nc.vector.reciprocal(rec, ps_o[HS:HS + 1, :])
                        ps_bc = ps_a.tile([P, 512], F32,
                                          name=f"psbc{h}_{qc}", tag="a")
                        nc.tensor.matmul(ps_bc[:HS, :], ones_sb[:, :HS], rec,
                                         start=True, stop=True)
                        bc = nrm.tile([HS, 512], F32, name=f"bc{h}_{qc}",
                                      tag="bc")
                        nc.vector.tensor_copy(bc, ps_bc[:HS, :])
                        nc.vector.tensor_mul(
                            outT[pb:pb + HS, j, qc * 512:(qc + 1) * 512],
                            ps_o[0:HS, :], bc)

        # ================= Phase P: output projection =================
        with tc.tile_pool(name="ypool", bufs=3) as yp:
            for tb in range(TO):
                for n2 in range(C // 512):
                    ps = ps_mm.tile([P, 512], F32, name=f"y_ps{tb}_{n2}",
                                    tag="mm")
                    for co in range(CO):
                        nc.tensor.matmul(
                            ps,
                            outT[:, co, tb * P:(tb + 1) * P],
                            wproj_sb[:, co, n2 * 512:(n2 + 1) * 512],
                            start=(co == 0), stop=(co == CO - 1))
                    y_sb = yp.tile([P, 512], F32, name=f"y_sb{tb}_{n2}",
                                   tag="y")
                    nc.vector.tensor_add(y_sb, ps,
                                         bproj_bc[:, n2 * 512:(n2 + 1) * 512])
                    nc.sync.dma_start(
                        y_r[:, tb, n2 * 512:(n2 + 1) * 512], y_sb)


_NC_CACHE = None


def _get_nc():
    global _NC_CACHE
    if _NC_CACHE is None:
        _NC_CACHE = build_nc()
    return _NC_CACHE


def kernel(x, W_qkv, b_qkv, W_proj, b_proj):
    """Full-input entry point: shards batch across 8 cores, returns [B,T,C]."""
    global LAST_RESULTS
    x = np.asarray(x, dtype=np.float32)
    W_qkv = np.asarray(W_qkv, dtype=np.float32)
    b_qkv = np.asarray(b_qkv, dtype=np.float32)
    W_proj = np.asarray(W_proj, dtype=np.float32)
    b_proj = np.asarray(b_proj, dtype=np.float32)

    scale = 1.0 / np.sqrt(HS)
    wqk = W_qkv[:, :2 * C].copy()
    wqk[:, :C] *= scale
    bqk = b_qkv[:2 * C].copy()
    bqk[:C] *= scale
    wv = np.ascontiguousarray(W_qkv[:, 2 * C:])
    bv = np.ascontiguousarray(b_qkv[2 * C:])
    ident = np.eye(P, dtype=np.float32)
    # mask[k, q] = 1 where q >= k (valid, causal), else 0
    mask = np.triu(np.ones((P, P), dtype=np.float32)).astype(ml_dtypes.bfloat16)

    ones = np.ones((1, P), dtype=np.float32)
    common = dict(wqk=wqk, wv=wv, wproj=W_proj, bqk=bqk, bv=bv,
                  bproj=b_proj, ident=ident, ones=ones, mask=mask)
    in_maps = [dict(x=np.ascontiguousarray(x[b]), **common) for b in range(B)]

    nc = _get_nc()
    res = run_bass_kernel_spmd(nc, in_maps, core_ids=list(range(B)),
                               trace=TRACE)
    LAST_RESULTS = res
    y = np.stack([res.results[b]["y"] for b in range(B)], axis=0)
    return y


# revision 16
# speedup vs baseline: 1.4569x; 1.4569x over previous
"""Causal self-attention Bass/Tile kernel for Trainium2 (8 NeuronCores).

Problem: y = CausalSelfAttention(x) with
  B=8, T=1024, C=1024, H=16 heads, hs=64.
  qkv = x @ W_qkv + b_qkv;  per-head causal softmax(q k^T / sqrt(hs)) @ v;
  y = out @ W_proj + b_proj.

Sharding: pure data parallel - core i computes batch element i end-to-end.
No collectives.

Per-core plan (fp32r matmuls except P·V which is bf16):
  1. Load x[b] [T,C] natural, PE-transpose 128x128 blocks -> xT [C,T].
  2. qkT [2C,T] = (W_qk)^T x^T via matmuls (lhsT = W chunk, rhs = xT), with
     the 1/sqrt(hs) score scale pre-folded into W_q/b_q on the host.
  3. v [T,C] natural via matmuls (lhsT = xT chunk, rhs = W_v), stored bf16
     into v_pad [T, kb, h, 65] whose 65th column is ones (fused row-sum).
  4. Scores TRANSPOSED: S^T[k,q] tile = matmul(lhsT=kT chunk, rhs=qT), two
     heads packed onto PE row-groups (K=64 each) via tile_position.
     One wide exp per (head, key-block) on ACT straight out of a 2-bank
     PSUM tile (no max subtraction needed; scores are O(1) by
     construction), bf16 out. Causal mask = multiplicative upper-tri mask
     on the diagonal 128x128 block only.
  5. PV: outT[h] [65, q] += matmul(lhsT=v_pad[:,kb,h,:], rhs=P^T tiles).
     Row 64 = softmax denominator s. Normalize: copy s row to SBUF,
     partition-broadcast via a K=1 ones matmul, reciprocal_approx_fast,
     multiply during the PSUM->SBUF copy of outT.
  6. proj: y [T,C] = matmul(lhsT=outT chunk, rhs=W_proj) + b_proj.

Emission is software-pipelined across head-pairs (qkT pair j+1 and v
halves interleave with attention of pair j) so the PE never idles long
enough for the HAM clock-gate to re-throttle it to 1.2 GHz.
"""

import os
from contextlib import ExitStack

import numpy as np
import ml_dtypes

import concourse.bass as bass
import concourse.bacc as bacc
import concourse.mybir as mybir
import concourse.tile as tile
from concourse.bass_utils import run_bass_kernel_spmd

F32 = mybir.dt.float32
F32R = mybir.dt.float32r
BF16 = mybir.dt.bfloat16

P = 128
B = 8
T = 1024
C = 1024
H = 16
HS = 64
TO = T // P   # 8 t-blocks
CO = C // P   # 8 c-chunks
NPAIR = H // 2  # 8 head pairs

# module-level knobs for test.py
TRACE = bool(int(os.environ.get("KERNEL_TRACE", "0")))
LAST_RESULTS = None  # BassKernelResults of last run


def build_nc():
    nc = bacc.Bacc("TRN2", target_bir_lowering=False, debug=False)

    x_d = nc.dram_tensor("x", [T, C], F32, kind="ExternalInput").ap()
    wqk_d = nc.dram_tensor("wqk", [C, 2 * C], F32R, kind="ExternalInput").ap()
    wv_d = nc.dram_tensor("wv", [C, C], F32R, kind="ExternalInput").ap()
    wproj_d = nc.dram_tensor("wproj", [C, C], F32R, kind="ExternalInput").ap()
    bqk_d = nc.dram_tensor("bqk", [2 * C], F32, kind="ExternalInput").ap()
    bv_d = nc.dram_tensor("bv", [C], F32R, kind="ExternalInput").ap()
    bproj_d = nc.dram_tensor("bproj", [C], F32R, kind="ExternalInput").ap()
    ident_d = nc.dram_tensor("ident", [P, P], F32, kind="ExternalInput").ap()
    ones_d = nc.dram_tensor("ones", [1, P], F32R, kind="ExternalInput").ap()
    mask_d = nc.dram_tensor("mask", [P, P], BF16, kind="ExternalInput").ap()
    y_d = nc.dram_tensor("y", [T, C], F32, kind="ExternalOutput").ap()

    with tile.TileContext(nc) as tc:
        _attn_body(tc, x_d, wqk_d, wv_d, wproj_d, bqk_d, bv_d, bproj_d,
                   ident_d, ones_d, mask_d, y_d)
    nc.compile()
    return nc


def _attn_body(tc, x_d, wqk_d, wv_d, wproj_d, bqk_d, bv_d, bproj_d,
               ident_d, ones_d, mask_d, y_d):
    nc = tc.nc
    with ExitStack() as ctx:
        # ---- pools that live the whole kernel ----
        consts = ctx.enter_context(tc.tile_pool(name="consts", bufs=1))
        big = ctx.enter_context(tc.tile_pool(name="big", bufs=1))
        ps_mm = ctx.enter_context(tc.tile_pool(name="ps_mm", bufs=2, space="PSUM"))

        # ---- constants ----
        ident_sb = consts.tile([P, P], F32, name="ident_sb")
        nc.sync.dma_start(ident_sb, ident_d)
        mask_sb = consts.tile([P, P], BF16, name="mask_sb")
        nc.sync.dma_start(mask_sb, mask_d)
        bqk_sb = consts.tile([P, 2 * C // P], F32, name="bqk_sb")
        nc.sync.dma_start(bqk_sb, bqk_d.rearrange("(m p) -> p m", p=P))
        ones_sb = consts.tile([1, P], F32R, name="ones_sb")
        nc.sync.dma_start(ones_sb, ones_d)
        rows_pool = tc.alloc_tile_pool(name="rows_pool", bufs=1)
        bv_row = rows_pool.tile([1, C], F32R, name="bv_row")
        nc.sync.dma_start(bv_row, bv_d[None, :])
        bproj_row = rows_pool.tile([1, C], F32R, name="bproj_row")
        nc.sync.dma_start(bproj_row, bproj_d[None, :])
        bv_bc = consts.tile([P, C], F32, name="bv_bc")
        bproj_bc = consts.tile([P, C], F32, name="bproj_bc")

        # ---- resident activations ----
        qkT = big.tile([P, 2 * C // P, T], F32R, name="qkT")  # 64KB/part
        v_pad = big.tile([P, TO, H, HS + 1], BF16, name="v_pad")  # 16.6KB/part
        outT = big.tile([P, CO, T], F32R, name="outT")      # 32KB/part

        x_r = x_d.rearrange("(to p) c -> p to c", p=P)
        y_r = y_d.rearrange("(tb p) c -> p tb c", p=P)
        wqk_r = wqk_d.rearrange("(co p) r -> p co r", p=P)
        wv_r = wv_d.rearrange("(co p) n -> p co n", p=P)
        wproj_r = wproj_d.rearrange("(co p) n -> p co n", p=P)

        # ============ Phase T: bias broadcasts, load x, transpose ============
        ps_tr = tc.alloc_tile_pool(name="ps_tr", bufs=2, space="PSUM")
        for n2 in range(C // 512):
            for row, dst in ((bv_row, bv_bc), (bproj_row, bproj_bc)):
                ps_b = ps_tr.tile([P, 512], F32, name=f"ps_b{n2}", tag="tr")
                nc.tensor.matmul(ps_b, ones_sb, row[:, n2 * 512:(n2 + 1) * 512],
                                 start=True, stop=True)
                nc.vector.tensor_copy(dst[:, n2 * 512:(n2 + 1) * 512], ps_b)
        rows_pool.release()
        xT_pool = tc.alloc_tile_pool(name="xT_pool", bufs=1)
        xT = xT_pool.tile([P, CO, T], F32R, name="xT")      # 32KB/part

        with tc.tile_pool(name="xload", bufs=3) as xload:
            for to in range(TO):
                x_t = xload.tile([P, C], F32, name=f"x_t{to}", tag="xt")
                nc.sync.dma_start(x_t, x_r[:, to, :])
                for co in range(CO):
                    pst = ps_tr.tile([P, 512], F32, name=f"pst{to}_{co}",
                                     tag="tr")
                    nc.tensor.transpose(
                        pst[:, :P], x_t[:, co * P:(co + 1) * P], ident_sb)
                    nc.vector.tensor_copy(
                        xT[:, co, to * P:(to + 1) * P], pst[:, :P])
        ps_tr.release()

        # attention-phase pools (released before the projection phase)
        attn_ctx = ExitStack()
        ps_sc = attn_ctx.enter_context(tc.tile_pool(name="ps_sc", bufs=2, space="PSUM"))
        ps_pv = attn_ctx.enter_context(tc.tile_pool(name="ps_pv", bufs=2, space="PSUM"))
        ptp = attn_ctx.enter_context(tc.tile_pool(name="pt_pool", bufs=2))
        nrm = attn_ctx.enter_context(tc.tile_pool(name="nrm", bufs=2))
        wqkp = attn_ctx.enter_context(tc.tile_pool(name="wqk_pool", bufs=2))
        wvp = attn_ctx.enter_context(tc.tile_pool(name="wv_pool", bufs=1))

        nc.vector.memset(v_pad[:, :, :, HS:HS + 1], 1.0)

        def emit_qkT(m):
            """qkT rows m*128..m*128+127 (transposed): lhsT=W chunk, rhs=xT."""
            w_m = wqkp.tile([P, CO, P], F32R, name=f"w_m{m}", tag="wqk")
            nc.sync.dma_start(w_m, wqk_r[:, :, m * P:(m + 1) * P])
            for n2 in range(T // 512):
                ps = ps_mm.tile([P, 512], F32, name=f"qk_ps{m}_{n2}", tag="mm")
                for co in range(CO):
                    nc.tensor.matmul(
                        ps, w_m[:, co, :],
                        xT[:, co, n2 * 512:(n2 + 1) * 512],
                        start=(co == 0), stop=(co == CO - 1))
                nc.vector.tensor_scalar_add(
                    qkT[:, m, n2 * 512:(n2 + 1) * 512], ps, bqk_sb[:, m:m + 1])

        def emit_v_half(n2):
            """v columns n2*512..: heads 8*n2..8*n2+7, all t, bf16 + bias."""
            wv_sb = wvp.tile([P, CO, 512], F32R, name=f"wv_sb{n2}", tag="wv")
            nc.sync.dma_start(wv_sb, wv_r[:, :, n2 * 512:(n2 + 1) * 512])
            for tb in range(TO):
                ps = ps_mm.tile([P, 512], F32, name=f"v_ps{tb}_{n2}", tag="mm")
                for co in range(CO):
                    nc.tensor.matmul(
                        ps, xT[:, co, tb * P:(tb + 1) * P],
                        wv_sb[:, co, :],
                        start=(co == 0), stop=(co == CO - 1))
                nc.vector.tensor_tensor(
                    out=v_pad[:, tb, n2 * 8:(n2 + 1) * 8, 0:HS],
                    in0=ps.rearrange("p (h d) -> p h d", d=HS),
                    in1=bv_bc[:, n2 * 512:(n2 + 1) * 512].rearrange(
                        "p (h d) -> p h d", d=HS),
                    op=mybir.AluOpType.add)

        def emit_scores(j):
            """S^T + exp + mask for both heads of pair j. Returns pt tiles."""
            pts = {}
            for hh in range(2):
                h = 2 * j + hh
                pb = hh * HS
                m_q, m_k = j, NPAIR + j
                for kb in range(TO):
                    w = T - kb * P
                    ps = ps_sc.tile([P, 1024], F32, name=f"s{h}_{kb}",
                                    tag="sc")
                    kT = qkT[pb:pb + HS, m_k, kb * P:(kb + 1) * P]
                    # matmul in <=512 chunks (fp32r moving-dim limit),
                    # all into one wide psum tile
                    off = 0
                    while off < w:
                        cw = min(512, w - off)
                        qs = kb * P + off
                        nc.tensor.matmul(
                            ps[:, off:off + cw], kT,
                            qkT[pb:pb + HS, m_q, qs:qs + cw],
                            start=True, stop=True, tile_position=(pb, 0))
                        off += cw
                    pt_kb = ptp.tile([P, w], BF16, name=f"pt{h}_{kb}",
                                     tag=f"pt{kb}")
                    nc.scalar.activation(
                        out=pt_kb, in_=ps[:, :w],
                        func=mybir.ActivationFunctionType.Exp)
                    nc.vector.tensor_mul(pt_kb[:, 0:P], pt_kb[:, 0:P], mask_sb)
                    pts[(hh, kb)] = pt_kb
            return pts

        def emit_pv(j, pts):
            """PV + row-sum + normalization into outT for both heads."""
            for hh in range(2):
                h = 2 * j + hh
                pb = hh * HS
                for qc in range(T // 512):
                    ps_o = ps_pv.tile([HS + 1, 512], F32, name=f"o{h}_{qc}",
                                      tag="pv")
                    kbs = [kb for kb in range(TO) if kb * P < (qc + 1) * 512]
                    for i, kb in enumerate(kbs):
                        qlo = max(qc * 512, kb * P)
                        qhi = (qc + 1) * 512
                        nc.tensor.matmul(
                            ps_o[:, qlo - qc * 512:512],
                            v_pad[:, kb, h, :],
                            pts[(hh, kb)][:, qlo - kb * P:qhi - kb * P],
                            start=(i == 0), stop=(i == len(kbs) - 1))
                    # normalization: s row -> sbuf, ones-matmul broadcast,
                    # fast reciprocal, multiply during psum->sbuf copy
                    srow = nrm.tile([1, 512], F32R, name=f"sr{h}_{qc}",
                                    tag="srow")
                    nc.vector.tensor_copy(srow, ps_o[HS:HS + 1, :])
                    ps_bc = ps_sc.tile([P, 1024], F32, name=f"psbc{h}_{qc}",
                                       tag="sc")
                    nc.tensor.matmul(ps_bc[:HS, :512], ones_sb[:, :HS], srow,
                                     start=True, stop=True)
                    bc = nrm.tile([HS, 512], F32, name=f"bc{h}_{qc}", tag="bc")
                    nc.vector.reciprocal_approx_fast(bc, ps_bc[:HS, :512])
                    nc.vector.tensor_mul(
                        outT[pb:pb + HS, j, qc * 512:(qc + 1) * 512],
                        ps_o[0:HS, :], bc)

        # ============ pipelined main loop ============
        emit_qkT(0)
        emit_qkT(NPAIR + 0)
        pts = emit_scores(0)
        emit_v_half(0)
        for j in range(NPAIR):
            if j + 1 < NPAIR:
                emit_qkT(j + 1)
                emit_qkT(NPAIR + j + 1)
            emit_pv(j, pts)
            if j == 1:
                emit_v_half(1)
            if j + 1 < NPAIR:
                pts = emit_scores(j + 1)

        # ============ Phase P: output projection ============
        attn_ctx.close()
        xT_pool.release()
        with tc.tile_pool(name="ypool", bufs=3) as yp, \
             tc.tile_pool(name="wprojp", bufs=2) as wpp:
            for n2 in range(C // 512):
                wproj_sb = wpp.tile([P, CO, 512], F32R, name=f"wproj{n2}",
                                    tag="wproj")
                nc.sync.dma_start(wproj_sb,
                                  wproj_r[:, :, n2 * 512:(n2 + 1) * 512])
                for tb in range(TO):
                    ps = ps_mm.tile([P, 512], F32, name=f"y_ps{tb}_{n2}",
                                    tag="mm")
                    for co in range(CO):
                        nc.tensor.matmul(
                            ps, outT[:, co, tb * P:(tb + 1) * P],
                            wproj_sb[:, co, :],
                            start=(co == 0), stop=(co == CO - 1))
                    y_sb = yp.tile([P, 512], F32, name=f"y_sb{tb}_{n2}",
                                   tag="y")
                    nc.vector.tensor_add(y_sb, ps,
                                         bproj_bc[:, n2 * 512:(n2 + 1) * 512])
                    nc.sync.dma_start(
                        y_r[:, tb, n2 * 512:(n2 + 1) * 512], y_sb)


_NC_CACHE = None


def _get_nc():
    global _NC_CACHE
    if _NC_CACHE is None:
        _NC_CACHE = build_nc()
    return _NC_CACHE


def kernel(x, W_qkv, b_qkv, W_proj, b_proj):
    """Full-input entry point: shards batch across 8 cores, returns [B,T,C]."""
    global LAST_RESULTS
    x = np.asarray(x, dtype=np.float32)
    W_qkv = np.asarray(W_qkv, dtype=np.float32)
    b_qkv = np.asarray(b_qkv, dtype=np.float32)
    W_proj = np.asarray(W_proj, dtype=np.float32)
    b_proj = np.asarray(b_proj, dtype=np.float32)

    scale = 1.0 / np.sqrt(HS)
    wqk = W_qkv[:, :2 * C].copy()
    wqk[:, :C] *= scale
    bqk = b_qkv[:2 * C].copy()
    bqk[:C] *= scale
    wv = np.ascontiguousarray(W_qkv[:, 2 * C:])
    bv = np.ascontiguousarray(b_qkv[2 * C:])
    ident = np.eye(P, dtype=np.float32)
    # mask[k, q] = 1 where q >= k (valid, causal), else 0
    mask = np.triu(np.ones((P, P), dtype=np.float32)).astype(ml_dtypes.bfloat16)
    ones = np.ones((1, P), dtype=np.float32)

    common = dict(wqk=wqk, wv=wv, wproj=W_proj, bqk=bqk, bv=bv,
                  bproj=b_proj, ident=ident, ones=ones, mask=mask)
    in_maps = [dict(x=np.ascontiguousarray(x[b]), **common) for b in range(B)]

    nc = _get_nc()
    res = run_bass_kernel_spmd(nc, in_maps, core_ids=list(range(B)),
                               trace=TRACE)
    LAST_RESULTS = res
    y = np.stack([res.results[b]["y"] for b in range(B)], axis=0)
    return y
